# revision 58
# baseline (speedup 1.0000x reference)
"""Causal attention kernel for Trainium2 (Bass/Tile), data-parallel over batch.

Problem (hardcoded): x[64,512,1024] f32, Wq/Wk/Wv[1024,256], bq/bk/bv[256].
  q = x@Wq+bq ; k = x@Wk+bk ; v = x@Wv+bv
  out = softmax(causal(q k^T / sqrt(256))) @ v           -> [64,512,256]

Sharding: 8 NeuronCores, 8 batches per core (pure data parallel, weights
replicated, no collectives). Each core runs the same program on its shard.

v2 design (vs the PE-transpose/fp32r v1):
  * x and W are cast to bf16 on the host; x is loaded ALREADY TRANSPOSED
    into SBUF via the DMA XBAR transpose (dma_start_transpose, 2-byte
    dtypes only) -- zero PE transposes for x.
  * All matmuls run bf16 (no fp32r small-free-dim penalty, FWL weight
    loads); PSUM accumulates fp32.
  * Transposed-scores formulation: scoresT[tk,tq] = kT_chunk.T @ qT, the
    causal mask + exp are applied in that layout, and the exp'd tile is
    used directly as the AV stationary: out[tq,d] = sum_s wT_s.T @ v_s.
    No transpose of the softmax weights is ever needed.
  * A ones-column appended to v makes the AV matmul emit the softmax
    row-sums for free (N=257); normalization is a reciprocal + ACT scale.
  * bq (pre-scaled by 1/sqrt(d)) and bk are folded into the ACT PSUM
    drains; bv is added at the end (softmax rows sum to 1).
"""

import numpy as np
import ml_dtypes

import concourse.bass as bass
import concourse.mybir as mybir
import concourse.tile as tile
from concourse import bacc
from concourse.bass_utils import run_bass_kernel_spmd

B, T, DM, D = 64, 512, 1024, 256
NCORES = 8
BPC = B // NCORES  # batches per core
P = 128
KO = DM // P  # 8 contraction subtiles for the projections
NCH = T // P  # 4 token chunks per sequence
DJ = D // P  # 2 head-dim chunks
VW = 260  # v row width: 256 d + 1 ones + 3 pad (8B-aligned rows)
SCALE = 1.0 / 16.0  # 256 ** -0.5
MASK_VAL = -1e30

F32 = mybir.dt.float32
BF16 = mybir.dt.bfloat16


def make_causal_mask_t(nc, out, mask_val):
    """Additive transposed-causal mask: out[i,j] = 0 if j >= i else mask_val.

    (For scoresT[tk, tq] diagonal blocks: valid iff tq >= tk.)"""
    sq = out.shape[0]
    nc.gpsimd.memset(out, 0.0)
    nc.gpsimd.affine_select(
        out=out,
        in_=out,
        compare_op=mybir.AluOpType.is_ge,
        fill=mask_val,
        base=0,
        # pred = -i + j >= 0  ->  keep 0 where valid, mask_val where j < i
        pattern=[[1, sq]],
        channel_multiplier=-1,
    )


def emit_core_program(ctx, nc: bass.Bass, tc, io, reps=1, hints=True,
                      v_drain_dve=True, out_gp=False, xq_split=True,
                      out_half=True, mm_bufs=3, s_bufs=3, qk_drain_dve=True,
                      ablate="none", staggered=True, bv_dve=True, dup=1,
                      body_b=BPC, out_bf16=True, av_bufs=2, av_lag=2,
                      attn2=True):
    x_d, wq_d, bq_d, wk_d, bk_d, wv_d, bv_d, out_d = io

    def enter_pool(name, bufs, space="SBUF"):
        return ctx.enter_context(tc.tile_pool(name=name, bufs=bufs, space=space))

    consts = enter_pool("consts", bufs=1)
    cmask = consts.tile([P, P], BF16, name="cmask")
    make_causal_mask_t(nc, cmask, MASK_VAL)
    ident = consts.tile([P, P], BF16, name="ident")
    from concourse.masks import make_identity
    make_identity(nc, ident)

    wq_s = consts.tile([P, KO, D], BF16, name="wq_s")
    wk_s = consts.tile([P, KO, D], BF16, name="wk_s")
    wv_s = consts.tile([P, KO, D], BF16, name="wv_s")
    bq16_s = consts.tile([P, DJ], F32, name="bq16_s")
    bk_s = consts.tile([P, DJ], F32, name="bk_s")
    bv_s = consts.tile([P, D], F32, name="bv_s")

    def load_consts_early():
        nc.scalar.dma_start(wq_s, wq_d.rearrange("(ko p) d -> p ko d", p=P))
        # bq arrives pre-scaled by 1/16 from the host
        nc.gpsimd.dma_start(bq16_s, bq_d.rearrange("(j p) -> p j", p=P))
        nc.gpsimd.dma_start(bk_s, bk_d.rearrange("(j p) -> p j", p=P))

    def load_consts_mid():
        nc.scalar.dma_start(wk_s, wk_d.rearrange("(ko p) d -> p ko d", p=P))

    def load_consts_late():
        nc.scalar.dma_start(wv_s, wv_d.rearrange("(ko p) d -> p ko d", p=P))
        nc.gpsimd.dma_start(bv_s, bv_d[None, :].to_broadcast((P, D)))

    xt_pool = enter_pool("xt", bufs=4)
    qk_pool = enter_pool("qk", bufs=4)
    v_pool = enter_pool("v", bufs=4)
    w_pool = enter_pool("w", bufs=2)
    o_pool = enter_pool("o", bufs=4)
    stat_pool = enter_pool("stat", bufs=8)
    # one shared PSUM ring for all projection matmuls (q/k/v), plus
    # dedicated rings for scores and AV: mm_bufs + 2 + 2 banks <= 8
    ps_mm = enter_pool("ps_mm", bufs=mm_bufs, space="PSUM")
    ps_s = enter_pool("ps_s", bufs=s_bufs, space="PSUM")
    ps_av = enter_pool("ps_av", bufs=av_bufs, space="PSUM")

    # consts load once, outside the timed hardware loop
    load_consts_early()
    load_consts_mid()
    load_consts_late()

    if reps > 1:
        he = (
            mybir.EngineType.PE, mybir.EngineType.DVE,
            mybir.EngineType.Activation, mybir.EngineType.SP,
        ) if hints else ()
        ctx.enter_context(tc.For_i(0, reps, 1, hint_engines=he,
                                   staggered_reset=staggered))

    class BatchCtx:
        def __init__(self, b):
            self.b = b
            self.xt = xt_pool.tile([P, KO, T], BF16, name="xt", tag="xt")
            self.qt = qk_pool.tile([P, DJ, T], BF16, name="qt", tag="qt")
            self.kt = qk_pool.tile([P, DJ, T], BF16, name="kt", tag="kt")
            self.v_sb = v_pool.tile([P, NCH, VW], BF16, name="v_sb", tag="v_sb")
            self.wts = [
                w_pool.tile([P, T], BF16, name="wt", tag=f"wt{s}")
                for s in range(NCH)
            ]
            self.oc = o_pool.tile(
                [P, NCH, D], BF16 if out_bf16 else F32, name="oc", tag="oc")

    def load_stage(bc, split=1):
        """DMA x[b] in as xT bf16 (pre-transposed on host: x_d is [B, DM, T];
        xt[p,ko,t] = xT[koP+p, t]). All loads ride the SP queue so they
        stream back-to-back, up to 4 batches ahead (xt ring depth); stores
        live on the ACT queue so a store waiting on attention never
        head-of-line-blocks a load."""
        src = x_d[bc.b].rearrange("(ko p) t -> p ko t", p=P)
        kstep = KO // split
        for k0 in range(0, KO, kstep):
            nc.sync.dma_start(bc.xt[:, k0:k0 + kstep, :], src[:, k0:k0 + kstep, :])
        nc.gpsimd.memset(bc.v_sb[:, :, D:D + 1], 1.0)

    def qk_group(bc, w_s, b_s, scl, j, which):
        """One (projection, j) group: 8-ko stationary chain + drain -> bf16."""
        pm = ps_mm.tile([P, T], F32, name="pm", tag="pm")
        for ko in range(KO):
            nc.tensor.matmul(
                pm,
                w_s[:, ko, j * P:(j + 1) * P],
                bc.xt[:, ko, :],
                start=(ko == 0),
                stop=(ko == KO - 1),
            )
        dst = bc.qt if which == "q" else bc.kt
        # split drains across DVE (q) and ACT (k) to balance engine load
        # (qk_drain_dve: 0 = both ACT, 1/True = q DVE + k ACT, 2 = both DVE)
        if qk_drain_dve and (which == "q" or qk_drain_dve == 2):
            nc.vector.tensor_scalar(
                dst[:, j, :], pm, scl, b_s[:, j:j + 1],
                op0=mybir.AluOpType.mult, op1=mybir.AluOpType.add,
            )
        else:
            nc.scalar.activation(
                dst[:, j, :], pm,
                mybir.ActivationFunctionType.Identity,
                bias=b_s[:, j:j + 1], scale=scl,
            )

    def v_group(bc, c):
        """v[tok chunk c, :]: stat = xT chunk, mov = Wv."""
        pv = ps_mm.tile([P, T], F32, name="pv", tag="pm")
        for ko in range(KO):
            nc.tensor.matmul(
                pv[:, :D],
                bc.xt[:, ko, c * P:(c + 1) * P],
                wv_s[:, ko, :],
                start=(ko == 0),
                stop=(ko == KO - 1),
            )
        if v_drain_dve:
            nc.vector.tensor_copy(bc.v_sb[:, c, :D], pv[:, :D])
        else:
            nc.scalar.copy(bc.v_sb[:, c, :D], pv[:, :D])

    def attention_stages(bc):
        def stage_s(s):
            n = T - s * P
            ps = ps_s.tile([P, T], F32, name="ps", tag="ps")
            # additive causal mask on the diagonal block (tq in [sP, sP+128))
            # folded into the PSUM accumulation: ps[:, :P] += I.T @ cmask
            nc.tensor.matmul(ps[:, :P], ident, cmask, start=True, stop=False)
            for j in range(DJ):
                nc.tensor.matmul(
                    ps[:, :n],
                    bc.kt[:, j, s * P:(s + 1) * P],
                    bc.qt[:, j, s * P:],
                    start=False,
                    stop=(j == DJ - 1),
                )
            nc.scalar.activation(
                bc.wts[s][:, :n], ps[:, :n], mybir.ActivationFunctionType.Exp,
            )

        def stage_av(c):
            po = ps_av.tile([P, T], F32, name="po", tag="pav")
            for s in range(c + 1):
                nc.tensor.matmul(
                    po[:, :D + 1],
                    bc.wts[s][:, (c - s) * P:(c - s) * P + P],
                    bc.v_sb[:, s, :D + 1],
                    start=(s == 0),
                    stop=(s == c),
                )
            linv = stat_pool.tile([P, 1], F32, name="linv", tag="linv")
            nc.vector.reciprocal(linv, po[:, D:D + 1])
            nc.scalar.activation(
                bc.oc[:, c, :], po[:, :D],
                mybir.ActivationFunctionType.Copy, scale=linv,
            )
            # tensor_tensor on DVE never takes the shared SBUF port pair,
            # so it can't block (or be blocked by) GpSimd
            eng = nc.vector if bv_dve else nc.gpsimd
            eng.tensor_add(bc.oc[:, c, :], bc.oc[:, c, :], bv_s)
            if c == NCH - 1:
                # one store per batch ([p, c, d] device layout, host
                # untransposes) on the store-only ACT queue
                nc.scalar.dma_start(out_d[bc.b], bc.oc)

        return [("s", stage_s, s) for s in range(NCH)], \
               [("av", stage_av, c) for c in range(NCH)]

    PROJS = (("q", wq_s, bq16_s, SCALE), ("k", wk_s, bk_s, 1.0))

    def proj_stages(bc):
        """qk + v projection emit-closures for one batch."""
        stages = []
        for which, w_s, b_s, scl in PROJS:
            for j in range(DJ):
                stages.append(
                    lambda which=which, w_s=w_s, b_s=b_s, scl=scl, j=j:
                    qk_group(bc, w_s, b_s, scl, j, which)
                )
        for c in range(NCH):
            stages.append(lambda c=c: v_group(bc, c))
        return stages

    def batch_prep(bc, first):
        """Emit-closures for loading + projecting one batch."""
        if "noload" in ablate and not first:
            return proj_stages(bc)
        return [lambda: load_stage(bc, split=2 if first else 1)] \
            + proj_stages(bc)

    # Cross-batch software pipeline: batch b's load/projections are emitted
    # riffled with batch b-1's attention stages so the PE always has
    # independent fill work during the softmax latencies.
    pending = None
    bc0 = None
    for bi in range(body_b * dup):
        b = bi % body_b
        bc = BatchCtx(b)
        if bi == 0:
            bc0 = bc
        elif "noload" in ablate:
            bc.xt = bc0.xt
        stages = batch_prep(bc, first=(bi == 0))
        fill = pending[0] if pending else []
        n = max(len(fill), len(stages))
        for i in range(n):
            if i < len(fill):
                _k, fn, c = fill[i]
                fn(c)
            if i < len(stages):
                stages[i]()
        ss, avs = attention_stages(bc)
        # AV lags its score stage by av_lag slots so the ACT exp latency is
        # hidden by later score matmuls (matters most in the epilogue):
        # e.g. lag 2: s0 s1 av0 s2 av1 s3 av2 av3
        merged = list(ss[:av_lag])
        for c in range(av_lag, NCH):
            merged += [avs[c - av_lag], ss[c]]
        merged += avs[NCH - av_lag:]
        if ablate.startswith("attn"):
            if "noout" not in ablate:
                nc.vector.memset(bc.oc, 0.0)
                nc.gpsimd.tensor_add(bc.oc[:, 0, :], bc.oc[:, 0, :], bv_s)
                nc.scalar.dma_start(out_d[bc.b], bc.oc)
            merged = []
        if attn2:
            # each batch's attention spans TWO prep periods: the fill for
            # prep(b+1) interleaves attn(b-1) 2nd half with attn(b) 1st half
            h = len(merged) // 2
            newest = (merged[:h], merged[h:])
            old_m2 = pending[1] if pending else []
            fill = []
            for i in range(max(len(old_m2), h)):
                if i < len(old_m2):
                    fill.append(old_m2[i])
                if i < h:
                    fill.append(newest[0][i])
            pending = (fill, newest[1])
        else:
            pending = (merged, [])
    for lst in pending:
        for _k, fn, c in lst:
            fn(c)


def build_program(reps=1, hints=True, **flags):
    """Build the single-core Bass program (same program runs on all 8 cores).

    reps > 1 wraps the whole body in a hardware loop (same work each
    iteration) -- used only for device-time measurement."""
    out_dt = BF16 if flags.get("out_bf16", True) else F32
    nc = bacc.Bacc("TRN2", target_bir_lowering=False, debug=False)
    x_d = nc.dram_tensor("x", [BPC, DM, T], BF16, kind="ExternalInput").ap()
    wq_d = nc.dram_tensor("wq", [DM, D], BF16, kind="ExternalInput").ap()
    bq_d = nc.dram_tensor("bq", [D], F32, kind="ExternalInput").ap()
    wk_d = nc.dram_tensor("wk", [DM, D], BF16, kind="ExternalInput").ap()
    bk_d = nc.dram_tensor("bk", [D], F32, kind="ExternalInput").ap()
    wv_d = nc.dram_tensor("wv", [DM, D], BF16, kind="ExternalInput").ap()
    bv_d = nc.dram_tensor("bv", [D], F32, kind="ExternalInput").ap()
    out_d = nc.dram_tensor(
        "out", [BPC, P, NCH, D], out_dt, kind="ExternalOutput").ap()

    from contextlib import ExitStack

    with tile.TileContext(nc) as tc, ExitStack() as ctx:
        emit_core_program(
            ctx, nc, tc, (x_d, wq_d, bq_d, wk_d, bk_d, wv_d, bv_d, out_d),
            reps=reps, hints=hints, **flags,
        )
    nc.compile()
    return nc


_NC_CACHE = None


def _get_program():
    global _NC_CACHE
    if _NC_CACHE is None:
        _NC_CACHE = build_program()
    return _NC_CACHE


def _bf16(a):
    return np.ascontiguousarray(np.asarray(a, np.float32)).astype(
        ml_dtypes.bfloat16)


def make_in_maps(inputs):
    # upload x already transposed ([B, DM, T]) so the device reads xT with
    # plain contiguous DMAs
    x = np.ascontiguousarray(_bf16(inputs["x"]).transpose(0, 2, 1))
    shared = {
        "wq": _bf16(inputs["Wq"]),
        # fold the 1/sqrt(d) score scaling into q's bias here; the matmul
        # part of the scale is applied in the ACT drain on-device
        "bq": np.ascontiguousarray(np.asarray(inputs["bq"], np.float32)) * SCALE,
        "wk": _bf16(inputs["Wk"]),
        "bk": np.ascontiguousarray(np.asarray(inputs["bk"], np.float32)),
        "wv": _bf16(inputs["Wv"]),
        "bv": np.ascontiguousarray(np.asarray(inputs["bv"], np.float32)),
    }
    return [
        {"x": x[i * BPC:(i + 1) * BPC], **shared} for i in range(NCORES)
    ]


def kernel(**inputs) -> np.ndarray:
    nc = _get_program()
    in_maps = make_in_maps(inputs)
    res = run_bass_kernel_spmd(nc, in_maps, core_ids=list(range(NCORES)))
    # device layout is [BPC, P, NCH, D] with token t = c*128 + p
    out = np.concatenate([m["out"] for m in res.results], axis=0)
    return np.ascontiguousarray(
        out.transpose(0, 2, 1, 3).reshape(B, T, D)).astype(np.float32)


# revision 59
# speedup vs baseline: 1.0746x; 1.0746x over previous
"""Causal attention kernel for Trainium2 (Bass/Tile), data-parallel over batch.

Problem (hardcoded): x[64,512,1024] f32, Wq/Wk/Wv[1024,256], bq/bk/bv[256].
  q = x@Wq+bq ; k = x@Wk+bk ; v = x@Wv+bv
  out = softmax(causal(q k^T / sqrt(256))) @ v           -> [64,512,256]

Sharding: 8 NeuronCores, 8 batches per core (pure data parallel, weights
replicated, no collectives). Each core runs the same program on its shard.

v2 design (vs the PE-transpose/fp32r v1; 220us -> ~150us):
  * x and W are cast to bf16 and x pre-transposed to [B, DM, T] on the
    host, so the device reads xT with ONE plain contiguous 1MB DMA per
    batch -- zero transposes on device. (The XBAR dma_start_transpose
    path was tried and is fast in theory, but loses a WAR race on real
    HW under pipelining -- corrupted batches at the DMA/compute
    crossover -- and HWDGE fixed cost makes many small DMAs ~2.4us each
    anyway: few mega-DMAs win.)
  * All matmuls run bf16 (no fp32r small-free-dim penalty, FWL weight
    loads are fully hidden even with per-MM stationary switches); PSUM
    accumulates fp32.
  * Transposed-scores formulation: scoresT[tk,tq] = kT_chunk.T @ qT, the
    causal mask + exp are applied in that layout, and the exp'd tile is
    used directly as the AV stationary: out[tq,d] = sum_s wT_s.T @ v_s.
    No transpose of the softmax weights is ever needed.
  * The additive causal mask of the diagonal block is folded into the
    scores PSUM accumulation as one extra matmul (I.T @ cmask) -- no
    DVE read-modify-write of PSUM on the softmax critical path.
  * A ones-column appended to v makes the AV matmul emit the softmax
    row-sums for free (N=257); normalization is a reciprocal + ACT scale.
  * bq (pre-scaled by 1/sqrt(d)) and bk are folded into the PSUM drains
    (q on DVE, k on ACT, to decouple the in-order engine queues); bv is
    added on DVE at the end (softmax rows sum to 1; DVE tensor_tensor
    never takes the GpSimd-shared SBUF port pair).
  * Batch-level software pipeline: batch b's load+projections are
    emitted riffled with older batches' attention stages; each batch's
    attention is spread over TWO prep periods (attn2) so two
    independent softmax chains are always in flight to fill PE stalls.
  * Loads all ride the SP HWDGE queue (streaming up to 4 batches ahead
    on the xt ring), stores the ACT queue -- a store waiting on
    attention results never head-of-line-blocks a load.
  * Output is stored bf16 in a device-friendly [B, P, NCH, D] layout
    (one DMA per batch); the host untransposes and upcasts.
"""

import numpy as np
import ml_dtypes

import concourse.bass as bass
import concourse.mybir as mybir
import concourse.tile as tile
from concourse import bacc
from concourse.bass_utils import run_bass_kernel_spmd

B, T, DM, D = 64, 512, 1024, 256
NCORES = 8
BPC = B // NCORES  # batches per core
P = 128
KO = DM // P  # 8 contraction subtiles for the projections
NCH = T // P  # 4 token chunks per sequence
DJ = D // P  # 2 head-dim chunks
VW = 260  # v row width: 256 d + 1 ones + 3 pad (8B-aligned rows)
SCALE = 1.0 / 16.0  # 256 ** -0.5
MASK_VAL = -1e30

F32 = mybir.dt.float32
BF16 = mybir.dt.bfloat16


def make_causal_mask_t(nc, out, mask_val):
    """Additive transposed-causal mask: out[i,j] = 0 if j >= i else mask_val.

    (For scoresT[tk, tq] diagonal blocks: valid iff tq >= tk.)"""
    sq = out.shape[0]
    nc.gpsimd.memset(out, 0.0)
    nc.gpsimd.affine_select(
        out=out,
        in_=out,
        compare_op=mybir.AluOpType.is_ge,
        fill=mask_val,
        base=0,
        # pred = -i + j >= 0  ->  keep 0 where valid, mask_val where j < i
        pattern=[[1, sq]],
        channel_multiplier=-1,
    )


def emit_core_program(ctx, nc: bass.Bass, tc, io, reps=1, hints=True,
                      v_drain_dve=True, out_gp=False, xq_split=True,
                      out_half=True, mm_bufs=3, s_bufs=3, qk_drain_dve=True,
                      ablate="none", staggered=True, bv_dve=True, dup=1,
                      body_b=BPC, out_bf16=True, av_bufs=2, av_lag=2,
                      attn2=True):
    x_d, wq_d, bq_d, wk_d, bk_d, wv_d, bv_d, out_d = io

    def enter_pool(name, bufs, space="SBUF"):
        return ctx.enter_context(tc.tile_pool(name=name, bufs=bufs, space=space))

    consts = enter_pool("consts", bufs=1)
    cmask = consts.tile([P, P], BF16, name="cmask")
    make_causal_mask_t(nc, cmask, MASK_VAL)
    ident = consts.tile([P, P], BF16, name="ident")
    from concourse.masks import make_identity
    make_identity(nc, ident)

    wq_s = consts.tile([P, KO, D], BF16, name="wq_s")
    wk_s = consts.tile([P, KO, D], BF16, name="wk_s")
    wv_s = consts.tile([P, KO, D], BF16, name="wv_s")
    bq16_s = consts.tile([P, DJ], F32, name="bq16_s")
    bk_s = consts.tile([P, DJ], F32, name="bk_s")
    bv_s = consts.tile([P, D], F32, name="bv_s")

    def load_consts_early():
        nc.scalar.dma_start(wq_s, wq_d.rearrange("(ko p) d -> p ko d", p=P))
        # bq arrives pre-scaled by 1/16 from the host
        nc.gpsimd.dma_start(bq16_s, bq_d.rearrange("(j p) -> p j", p=P))
        nc.gpsimd.dma_start(bk_s, bk_d.rearrange("(j p) -> p j", p=P))

    def load_consts_mid():
        nc.scalar.dma_start(wk_s, wk_d.rearrange("(ko p) d -> p ko d", p=P))

    def load_consts_late():
        nc.scalar.dma_start(wv_s, wv_d.rearrange("(ko p) d -> p ko d", p=P))
        nc.gpsimd.dma_start(bv_s, bv_d[None, :].to_broadcast((P, D)))

    xt_pool = enter_pool("xt", bufs=4)
    qk_pool = enter_pool("qk", bufs=4)
    v_pool = enter_pool("v", bufs=4)
    w_pool = enter_pool("w", bufs=2)
    o_pool = enter_pool("o", bufs=4)
    stat_pool = enter_pool("stat", bufs=8)
    # one shared PSUM ring for all projection matmuls (q/k/v), plus
    # dedicated rings for scores and AV: mm_bufs + 2 + 2 banks <= 8
    ps_mm = enter_pool("ps_mm", bufs=mm_bufs, space="PSUM")
    ps_s = enter_pool("ps_s", bufs=s_bufs, space="PSUM")
    ps_av = enter_pool("ps_av", bufs=av_bufs, space="PSUM")

    # consts load once, outside the timed hardware loop
    load_consts_early()
    load_consts_mid()
    load_consts_late()

    if reps > 1:
        he = (
            mybir.EngineType.PE, mybir.EngineType.DVE,
            mybir.EngineType.Activation, mybir.EngineType.SP,
        ) if hints else ()
        ctx.enter_context(tc.For_i(0, reps, 1, hint_engines=he,
                                   staggered_reset=staggered))

    class BatchCtx:
        def __init__(self, b):
            self.b = b
            self.xt = xt_pool.tile([P, KO, T], BF16, name="xt", tag="xt")
            self.qt = qk_pool.tile([P, DJ, T], BF16, name="qt", tag="qt")
            self.kt = qk_pool.tile([P, DJ, T], BF16, name="kt", tag="kt")
            self.v_sb = v_pool.tile([P, NCH, VW], BF16, name="v_sb", tag="v_sb")
            self.wts = [
                w_pool.tile([P, T], BF16, name="wt", tag=f"wt{s}")
                for s in range(NCH)
            ]
            self.oc = o_pool.tile(
                [P, NCH, D], BF16 if out_bf16 else F32, name="oc", tag="oc")

    def load_stage(bc, split=1):
        """DMA x[b] in as xT bf16 (pre-transposed on host: x_d is [B, DM, T];
        xt[p,ko,t] = xT[koP+p, t]). All loads ride the SP queue so they
        stream back-to-back, up to 4 batches ahead (xt ring depth); stores
        live on the ACT queue so a store waiting on attention never
        head-of-line-blocks a load."""
        src = x_d[bc.b].rearrange("(ko p) t -> p ko t", p=P)
        kstep = KO // split
        for k0 in range(0, KO, kstep):
            nc.sync.dma_start(bc.xt[:, k0:k0 + kstep, :], src[:, k0:k0 + kstep, :])
        nc.gpsimd.memset(bc.v_sb[:, :, D:D + 1], 1.0)

    def qk_group(bc, w_s, b_s, scl, j, which):
        """One (projection, j) group: 8-ko stationary chain + drain -> bf16."""
        pm = ps_mm.tile([P, T], F32, name="pm", tag="pm")
        for ko in range(KO):
            nc.tensor.matmul(
                pm,
                w_s[:, ko, j * P:(j + 1) * P],
                bc.xt[:, ko, :],
                start=(ko == 0),
                stop=(ko == KO - 1),
            )
        dst = bc.qt if which == "q" else bc.kt
        # split drains across DVE (q) and ACT (k) to balance engine load
        # (qk_drain_dve: 0 = both ACT, 1/True = q DVE + k ACT, 2 = both DVE)
        if qk_drain_dve and (which == "q" or qk_drain_dve == 2):
            nc.vector.tensor_scalar(
                dst[:, j, :], pm, scl, b_s[:, j:j + 1],
                op0=mybir.AluOpType.mult, op1=mybir.AluOpType.add,
            )
        else:
            nc.scalar.activation(
                dst[:, j, :], pm,
                mybir.ActivationFunctionType.Identity,
                bias=b_s[:, j:j + 1], scale=scl,
            )

    def v_group(bc, c):
        """v[tok chunk c, :]: stat = xT chunk, mov = Wv."""
        pv = ps_mm.tile([P, T], F32, name="pv", tag="pm")
        for ko in range(KO):
            nc.tensor.matmul(
                pv[:, :D],
                bc.xt[:, ko, c * P:(c + 1) * P],
                wv_s[:, ko, :],
                start=(ko == 0),
                stop=(ko == KO - 1),
            )
        if v_drain_dve:
            nc.vector.tensor_copy(bc.v_sb[:, c, :D], pv[:, :D])
        else:
            nc.scalar.copy(bc.v_sb[:, c, :D], pv[:, :D])

    def attention_stages(bc):
        def stage_s(s):
            n = T - s * P
            ps = ps_s.tile([P, T], F32, name="ps", tag="ps")
            # additive causal mask on the diagonal block (tq in [sP, sP+128))
            # folded into the PSUM accumulation: ps[:, :P] += I.T @ cmask
            nc.tensor.matmul(ps[:, :P], ident, cmask, start=True, stop=False)
            for j in range(DJ):
                nc.tensor.matmul(
                    ps[:, :n],
                    bc.kt[:, j, s * P:(s + 1) * P],
                    bc.qt[:, j, s * P:],
                    start=False,
                    stop=(j == DJ - 1),
                )
            nc.scalar.activation(
                bc.wts[s][:, :n], ps[:, :n], mybir.ActivationFunctionType.Exp,
            )

        def stage_av(c):
            po = ps_av.tile([P, T], F32, name="po", tag="pav")
            for s in range(c + 1):
                nc.tensor.matmul(
                    po[:, :D + 1],
                    bc.wts[s][:, (c - s) * P:(c - s) * P + P],
                    bc.v_sb[:, s, :D + 1],
                    start=(s == 0),
                    stop=(s == c),
                )
            linv = stat_pool.tile([P, 1], F32, name="linv", tag="linv")
            nc.vector.reciprocal(linv, po[:, D:D + 1])
            nc.scalar.activation(
                bc.oc[:, c, :], po[:, :D],
                mybir.ActivationFunctionType.Copy, scale=linv,
            )
            # tensor_tensor on DVE never takes the shared SBUF port pair,
            # so it can't block (or be blocked by) GpSimd
            eng = nc.vector if bv_dve else nc.gpsimd
            eng.tensor_add(bc.oc[:, c, :], bc.oc[:, c, :], bv_s)
            if c == NCH - 1:
                # one store per batch ([p, c, d] device layout, host
                # untransposes) on the store-only ACT queue
                nc.scalar.dma_start(out_d[bc.b], bc.oc)

        return [("s", stage_s, s) for s in range(NCH)], \
               [("av", stage_av, c) for c in range(NCH)]

    PROJS = (("q", wq_s, bq16_s, SCALE), ("k", wk_s, bk_s, 1.0))

    def proj_stages(bc):
        """qk + v projection emit-closures for one batch."""
        stages = []
        for which, w_s, b_s, scl in PROJS:
            for j in range(DJ):
                stages.append(
                    lambda which=which, w_s=w_s, b_s=b_s, scl=scl, j=j:
                    qk_group(bc, w_s, b_s, scl, j, which)
                )
        for c in range(NCH):
            stages.append(lambda c=c: v_group(bc, c))
        return stages

    def batch_prep(bc, first):
        """Emit-closures for loading + projecting one batch."""
        if "noload" in ablate and not first:
            return proj_stages(bc)
        return [lambda: load_stage(bc, split=2 if first else 1)] \
            + proj_stages(bc)

    # Cross-batch software pipeline: batch b's load/projections are emitted
    # riffled with batch b-1's attention stages so the PE always has
    # independent fill work during the softmax latencies.
    pending = None
    bc0 = None
    for bi in range(body_b * dup):
        b = bi % body_b
        bc = BatchCtx(b)
        if bi == 0:
            bc0 = bc
        elif "noload" in ablate:
            bc.xt = bc0.xt
        stages = batch_prep(bc, first=(bi == 0))
        fill = pending[0] if pending else []
        n = max(len(fill), len(stages))
        for i in range(n):
            if i < len(fill):
                _k, fn, c = fill[i]
                fn(c)
            if i < len(stages):
                stages[i]()
        ss, avs = attention_stages(bc)
        # AV lags its score stage by av_lag slots so the ACT exp latency is
        # hidden by later score matmuls (matters most in the epilogue):
        # e.g. lag 2: s0 s1 av0 s2 av1 s3 av2 av3
        merged = list(ss[:av_lag])
        for c in range(av_lag, NCH):
            merged += [avs[c - av_lag], ss[c]]
        merged += avs[NCH - av_lag:]
        if ablate.startswith("attn"):
            if "noout" not in ablate:
                nc.vector.memset(bc.oc, 0.0)
                nc.gpsimd.tensor_add(bc.oc[:, 0, :], bc.oc[:, 0, :], bv_s)
                nc.scalar.dma_start(out_d[bc.b], bc.oc)
            merged = []
        if attn2:
            # each batch's attention spans TWO prep periods: the fill for
            # prep(b+1) interleaves attn(b-1) 2nd half with attn(b) 1st half
            h = len(merged) // 2
            newest = (merged[:h], merged[h:])
            old_m2 = pending[1] if pending else []
            fill = []
            for i in range(max(len(old_m2), h)):
                if i < len(old_m2):
                    fill.append(old_m2[i])
                if i < h:
                    fill.append(newest[0][i])
            pending = (fill, newest[1])
        else:
            pending = (merged, [])
    for lst in pending:
        for _k, fn, c in lst:
            fn(c)


def build_program(reps=1, hints=True, **flags):
    """Build the single-core Bass program (same program runs on all 8 cores).

    reps > 1 wraps the whole body in a hardware loop (same work each
    iteration) -- used only for device-time measurement."""
    out_dt = BF16 if flags.get("out_bf16", True) else F32
    nc = bacc.Bacc("TRN2", target_bir_lowering=False, debug=False)
    x_d = nc.dram_tensor("x", [BPC, DM, T], BF16, kind="ExternalInput").ap()
    wq_d = nc.dram_tensor("wq", [DM, D], BF16, kind="ExternalInput").ap()
    bq_d = nc.dram_tensor("bq", [D], F32, kind="ExternalInput").ap()
    wk_d = nc.dram_tensor("wk", [DM, D], BF16, kind="ExternalInput").ap()
    bk_d = nc.dram_tensor("bk", [D], F32, kind="ExternalInput").ap()
    wv_d = nc.dram_tensor("wv", [DM, D], BF16, kind="ExternalInput").ap()
    bv_d = nc.dram_tensor("bv", [D], F32, kind="ExternalInput").ap()
    out_d = nc.dram_tensor(
        "out", [BPC, P, NCH, D], out_dt, kind="ExternalOutput").ap()

    from contextlib import ExitStack

    with tile.TileContext(nc) as tc, ExitStack() as ctx:
        emit_core_program(
            ctx, nc, tc, (x_d, wq_d, bq_d, wk_d, bk_d, wv_d, bv_d, out_d),
            reps=reps, hints=hints, **flags,
        )
    nc.compile()
    return nc


_NC_CACHE = None


def _get_program():
    global _NC_CACHE
    if _NC_CACHE is None:
        _NC_CACHE = build_program()
    return _NC_CACHE


def _bf16(a):
    return np.ascontiguousarray(np.asarray(a, np.float32)).astype(
        ml_dtypes.bfloat16)


def make_in_maps(inputs):
    # upload x already transposed ([B, DM, T]) so the device reads xT with
    # plain contiguous DMAs
    x = np.ascontiguousarray(_bf16(inputs["x"]).transpose(0, 2, 1))
    shared = {
        "wq": _bf16(inputs["Wq"]),
        # fold the 1/sqrt(d) score scaling into q's bias here; the matmul
        # part of the scale is applied in the ACT drain on-device
        "bq": np.ascontiguousarray(np.asarray(inputs["bq"], np.float32)) * SCALE,
        "wk": _bf16(inputs["Wk"]),
        "bk": np.ascontiguousarray(np.asarray(inputs["bk"], np.float32)),
        "wv": _bf16(inputs["Wv"]),
        "bv": np.ascontiguousarray(np.asarray(inputs["bv"], np.float32)),
    }
    return [
        {"x": x[i * BPC:(i + 1) * BPC], **shared} for i in range(NCORES)
    ]


def kernel(**inputs) -> np.ndarray:
    nc = _get_program()
    in_maps = make_in_maps(inputs)
    res = run_bass_kernel_spmd(nc, in_maps, core_ids=list(range(NCORES)))
    # device layout is [BPC, P, NCH, D] with token t = c*128 + p
    out = np.concatenate([m["out"] for m in res.results], axis=0)
    return np.ascontiguousarray(
        out.transpose(0, 2, 1, 3).reshape(B, T, D)).astype(np.float32)
